# revision 1
# baseline (speedup 1.0000x reference)
"""Trainium2 Bass kernel for HNet attention (B=4, S=2048, H=768, 12 heads,
RoPE, causal) — bf16 rework of the fp32r baseline.

Sharding: 8 cores = 4 batches x 2 head-groups (6 heads each).
Wq/Wk/Wv split column-wise (head axis), Wo row-wise; host sums the two
partial o_proj outputs per batch.

vs baseline:
  - all matmul operands bf16 (no fp32r <256 4x penalty, half the DMA)
  - PE transposes into bitcast spare regions of the ring psum tile
    (dma_start_transpose is NOT properly synced by the tile framework)
  - score kc-blocks in PAIRS into one [128,1024] psum; one exp per pair
  - flattened schedule: unit-level interleave of attention pairs,
    projections and o_proj so the in-order PE never reaches an
    instruction before its inputs are ready
  - 3-op RoPE (pre-swapped sin table folds rotate_half into the adds)
  - reciprocal at PV-sum extraction; attn_post deferred into next strip
  - bf16 output partials, host sums in fp32
"""

import os
import sys

import numpy as np

sys.path.insert(0, "/opt/trn_rl_repo")

from contextlib import ExitStack

import ml_dtypes

import concourse.bacc as bacc
import concourse.tile as tile
from concourse import mybir
from concourse.bass_utils import run_bass_kernel_spmd

S = 2048
HID = 768
NH = 6            # heads per core
D = 64
F = NH * D        # 384 per-core feature slice
P = 128
SC = S // P       # 16
FC = HID // P     # 6
MC = F // P       # 3
QW = 512          # q strip width
NQ = S // QW      # 4
N_CORES = 8
ROPE_THETA = 10000.0

F32 = mybir.dt.float32
F32R = mybir.dt.float32r
BF16 = mybir.dt.bfloat16
AF = mybir.ActivationFunctionType


def _h3(ap):
    """[P, F] -> [P, NH, D] view."""
    return ap.rearrange("p (h d) -> p h d", h=NH)


def build_program():
    nc = bacc.Bacc("TRN2", target_bir_lowering=False, debug=False,
                   num_devices=N_CORES)

    xT_d = nc.dram_tensor("xT", [HID, S], BF16, kind="ExternalInput").ap()
    wqkT_d = nc.dram_tensor("wqkT", [HID, 2 * F], BF16,
                            kind="ExternalInput").ap()
    wvT_d = nc.dram_tensor("wvT", [HID, F], BF16, kind="ExternalInput").ap()
    woT_d = nc.dram_tensor("woT", [F, HID], BF16, kind="ExternalInput").ap()
    cos_d = nc.dram_tensor("cos6", [S, F], BF16, kind="ExternalInput").ap()
    sin_d = nc.dram_tensor("sin6p", [S, F], BF16, kind="ExternalInput").ap()
    tri_d = nc.dram_tensor("tri", [P, P], BF16, kind="ExternalInput").ap()
    eye_d = nc.dram_tensor("eye", [P, P], F32R, kind="ExternalInput").ap()
    e_d = nc.dram_tensor("emat", [NH, F], F32R, kind="ExternalInput").ap()
    on_d = nc.dram_tensor("ones6", [P, NH], BF16, kind="ExternalInput").ap()
    out_d = nc.dram_tensor("out", [S, HID], BF16, kind="ExternalOutput").ap()

    with tile.TileContext(nc) as tc, ExitStack() as ctx:
        const_pool = ctx.enter_context(tc.tile_pool(name="const", bufs=1))
        tri_sb = const_pool.tile([P, P], BF16, tag="tri")
        nc.sync.dma_start(tri_sb[:], tri_d[:])
        eye_sb = const_pool.tile([P, P], F32R, tag="eye")
        nc.sync.dma_start(eye_sb[:], eye_d[:])
        e_sb = const_pool.tile([NH, F], F32R, tag="emat")
        nc.sync.dma_start(e_sb[:], e_d[:])
        on_sb = const_pool.tile([P, NH], BF16, tag="ones6")
        nc.sync.dma_start(on_sb[:], on_d[:])

        qkT_pool = ctx.enter_context(tc.tile_pool(name="qkT", bufs=1))
        kTb = qkT_pool.tile([P, MC, S], BF16, tag="kTb", name="kTb")
        kT = [kTb[:, m, :] for m in range(MC)]
        v_pool = ctx.enter_context(tc.tile_pool(name="vp", bufs=1))
        v_sb = [v_pool.tile([P, NH * 65], BF16, tag=f"v{s}", name=f"v{s}")
                for s in range(SC)]
        ao_pool = ctx.enter_context(tc.tile_pool(name="ao", bufs=2))
        woT_pool = ctx.enter_context(tc.tile_pool(name="woT", bufs=1))
        woT = [woT_pool.tile([P, HID], BF16, tag=f"woT{m}", name=f"woT{m}")
               for m in range(MC)]

        # PSUM (8 banks): ring 2x[P,1024 f32]=4 + pvp 2x[65,QW f32]=2
        #                 + pq,pk [P,F f32] = 2
        with tc.tile_pool(name="xT", bufs=1) as xT_pool, \
             tc.tile_pool(name="wT", bufs=1) as wT_pool, \
             tc.tile_pool(name="ld", bufs=6) as ld_pool, \
             tc.tile_pool(name="rope", bufs=3) as rope_pool, \
             tc.tile_pool(name="ex", bufs=10) as ex_pool, \
             tc.tile_pool(name="stg", bufs=6) as stg_pool, \
             tc.tile_pool(name="sums", bufs=2) as sums_pool, \
             tc.tile_pool(name="ob", bufs=6) as ob_pool, \
             tc.tile_pool(name="ps_qk", bufs=1, space="PSUM") as ps_qk, \
             tc.tile_pool(name="ring", bufs=2, space="PSUM") as ring, \
             tc.tile_pool(name="ps_pv", bufs=2, space="PSUM") as ps_pv:

            xT = [xT_pool.tile([P, S], BF16, tag=f"xT{f}", name=f"xT{f}")
                  for f in range(FC)]
            wqk = [wT_pool.tile([P, 2 * F], BF16, tag=f"wqk{f}",
                                name=f"wqk{f}") for f in range(FC)]
            wvT = [wT_pool.tile([P, F], BF16, tag=f"wvT{f}", name=f"wvT{f}")
                   for f in range(FC)]
            for f in range(FC):
                fs = slice(f * P, (f + 1) * P)
                nc.sync.dma_start(wvT[f][:], wvT_d[fs, :])
                nc.sync.dma_start(xT[f][:, 0:2 * QW], xT_d[fs, 0:2 * QW])
                nc.scalar.dma_start(wqk[f][:], wqkT_d[fs, :])
            for f in range(FC):
                fs = slice(f * P, (f + 1) * P)
                nc.sync.dma_start(xT[f][:, 2 * QW:S], xT_d[fs, 2 * QW:S])
            for m in range(MC):
                nc.scalar.dma_start(woT[m][:], woT_d[m * P:(m + 1) * P, :])
            # preload the Exp activation table during the DMA warmup
            warm = ld_pool.tile([1, 1], BF16, tag="warm", name="warm")
            nc.scalar.activation(warm[:], on_sb[0:1, 0:1], AF.Exp, scale=1.0)

            def emit_vproj(s):
                sl = slice(s * P, (s + 1) * P)
                pv_ = ring.tile([P, 2 * QW], F32, tag="ring", name="pv")
                for f in range(FC):
                    nc.tensor.matmul(pv_[:, 0:F], xT[f][:, sl], wvT[f][:],
                                     start=(f == 0), stop=(f == FC - 1))
                v3 = v_sb[s].rearrange("p (h e) -> p h e", h=NH)
                nc.scalar.copy(v3[:, :, 0:64], _h3(pv_[:, 0:F]))
                nc.gpsimd.tensor_copy(
                    v3[:, :, 64:65],
                    on_sb.rearrange("p (h o) -> p h o", h=NH))

            def emit_proj(s):
                sl = slice(s * P, (s + 1) * P)
                cs = ld_pool.tile([P, F], BF16, tag="cos", name="cs")
                nc.scalar.dma_start(cs[:], cos_d[sl, :])
                sn = ld_pool.tile([P, F], BF16, tag="sin", name="sn")
                nc.scalar.dma_start(sn[:], sin_d[sl, :])
                pq = ps_qk.tile([P, F], F32, tag="pq", name="pq")
                pk = ps_qk.tile([P, F], F32, tag="pk", name="pk")
                for f in range(FC):     # all q matmuls first: RoPE-q can
                    nc.tensor.matmul(pq[:], xT[f][:, sl], wqk[f][:, 0:F],
                                     start=(f == 0), stop=(f == FC - 1))
                for f in range(FC):     # start while the k chain runs
                    nc.tensor.matmul(pk[:], xT[f][:, sl], wqk[f][:, F:2 * F],
                                     start=(f == 0), stop=(f == FC - 1))
                return s, sl, cs, sn, pq, pk

            def emit_tail(state, qTb):
                s, sl, cs, sn, pq, pk = state
                qcol = (s % 4) * P
                # RoPE: qr = pp*cos + rot_half(pp)*sin_signed.  sin table is
                # host-pre-swapped (snp[d] = sin_signed[(d+32)%64]) so the
                # rotate_half becomes cross-half adds.
                for half in range(2):
                    pp = pq[:] if half == 0 else pk[:]
                    t1 = rope_pool.tile([P, F], BF16, tag="t1", name="t1")
                    nc.vector.tensor_mul(t1[:], pp, cs[:])
                    t2 = rope_pool.tile([P, F], BF16, tag="t2", name="t2")
                    nc.vector.tensor_mul(t2[:], pp, sn[:])
                    qr = rope_pool.tile([P, F], F32R, tag="qr", name="qr")
                    nc.vector.tensor_add(_h3(qr)[:, :, 0:32],
                                         _h3(t1)[:, :, 0:32],
                                         _h3(t2)[:, :, 32:64])
                    nc.vector.tensor_add(_h3(qr)[:, :, 32:64],
                                         _h3(t1)[:, :, 32:64],
                                         _h3(t2)[:, :, 0:32])
                    ptile = ring.tile([P, 2 * QW], F32, tag="ring",
                                      name="pt")
                    for m in range(MC):
                        ptv = ptile[:, m * P:(m + 1) * P].bitcast(F32R)
                        nc.tensor.transpose(ptv, qr[:, m * P:(m + 1) * P],
                                            eye_sb[:])
                        dst = (qTb[:, m, qcol:qcol + P] if half == 0
                               else kTb[:, m, sl])
                        nc.vector.tensor_copy(dst, ptv)

            def attn_unit(qc, m, t, qTs, pvps):
                q0 = qc * QW
                last = 4 * qc + 3
                kc0, kc1 = t, t + 1
                qlo0, qlo1 = max(q0, kc0 * P), max(q0, kc1 * P)
                n0, n1 = q0 + QW - qlo0, q0 + QW - qlo1
                exs = []
                for par in range(2):
                    off = 64 * par
                    sp = ring.tile([P, 2 * QW], F32, tag="ring", name="sp")
                    nc.tensor.matmul(sp[:, 0:n0],
                                     kT[m][off:off + 64, kc0 * P:kc0 * P + P],
                                     qTs[m][off:off + 64,
                                            qlo0 - q0:qlo0 - q0 + n0],
                                     start=True, stop=True)
                    nc.tensor.matmul(sp[:, n0:n0 + n1],
                                     kT[m][off:off + 64, kc1 * P:kc1 * P + P],
                                     qTs[m][off:off + 64,
                                            qlo1 - q0:qlo1 - q0 + n1],
                                     start=True, stop=True)
                    ex = ex_pool.tile([P, 2 * QW], BF16, tag="ex", name="ex")
                    nc.scalar.activation(ex[:, 0:n0 + n1], sp[:, 0:n0 + n1],
                                         AF.Exp, scale=0.125)
                    if kc0 * P >= q0:   # both blocks diagonal: mask
                        nc.vector.tensor_mul(ex[:, 0:P], ex[:, 0:P],
                                             tri_sb[:])
                        nc.gpsimd.tensor_mul(ex[:, n0:n0 + P],
                                             ex[:, n0:n0 + P], tri_sb[:])
                    exs.append(ex)
                for par in range(2):
                    h = 2 * m + par
                    ex = exs[par]
                    nc.tensor.matmul(pvps[par][:, qlo0 - q0:QW],
                                     v_sb[kc0][:, h * 65:h * 65 + 65],
                                     ex[:, 0:n0],
                                     start=(kc0 == 0), stop=False)
                    nc.tensor.matmul(pvps[par][:, qlo1 - q0:QW],
                                     v_sb[kc1][:, h * 65:h * 65 + 65],
                                     ex[:, n0:n0 + n1],
                                     start=False, stop=(kc1 == last))

            def pair_end(qc, m, aoT, sums, pvps):
                for par in range(2):
                    h = 2 * m + par
                    stg = stg_pool.tile([65, QW], F32R, tag="stg", name="stg")
                    with nc.allow_low_precision(reason="softmax sums"):
                        nc.vector.reciprocal(stg[64:65, :],
                                             pvps[par][64:65, :])
                    nc.sync.dma_start(sums[h:h + 1, :], stg[64:65, :])
                for par in range(2):
                    off = 64 * par
                    nc.vector.tensor_copy(aoT[m][off:off + 64, :],
                                          pvps[par][0:64, :])

            def emit_bp(m, aoT, sums):
                bp = ring.tile([P, 2 * QW], F32, tag="ring", name="bp")
                nc.tensor.matmul(bp[:, 0:QW], e_sb[:, m * P:(m + 1) * P],
                                 sums[:], start=True, stop=True)
                nc.vector.tensor_mul(aoT[m][:, :], aoT[m][:, :], bp[:, 0:QW])

            def attn_post(qc, aoT, sums):
                q0 = qc * QW
                for m in range(MC):
                    emit_bp(m, aoT, sums)
                for t in range(QW // P):
                    s0 = q0 + t * P
                    fin = ring.tile([P, 2 * QW], F32, tag="ring", name="fin")
                    for half in range(2):
                        c0 = half * F
                        o0 = half * QW      # bank-aligned psum offset
                        for m in range(MC):
                            nc.tensor.matmul(fin[:, o0:o0 + F],
                                             aoT[m][:, s0 - q0:s0 - q0 + P],
                                             woT[m][:, c0:c0 + F],
                                             start=(m == 0),
                                             stop=(m == MC - 1))
                    ob = ob_pool.tile([P, HID], BF16, tag="ob", name="ob")
                    fin2 = fin.rearrange("p (c x) -> p c x", c=2)
                    nc.vector.tensor_copy(
                        ob.rearrange("p (c x) -> p c x", c=2)[:, :, :],
                        fin2[:, :, 0:F])
                    nc.sync.dma_start(out_d[s0:s0 + P, :], ob[:])

            def alloc_strip(qc):
                qTb = rope_pool.tile([P, MC, QW], BF16, tag="qTb",
                                     name="qTb")
                qTs = [qTb[:, m, :] for m in range(MC)]
                aoT = [ao_pool.tile([P, QW], BF16, tag=f"aoTs{m}",
                                    name=f"aoTs{m}") for m in range(MC)]
                sums = sums_pool.tile([NH, QW], F32R, tag="sums",
                                      name="sums")
                return qTb, qTs, aoT, sums

            # ---- schedule ----
            strips = {0: alloc_strip(0)}
            # prologue: V projections interleaved with strip-0 projections
            for s in range(6):
                emit_vproj(s)
            vnext = 6
            for i in range(4):
                st = emit_proj(i)
                if vnext < SC:
                    emit_vproj(vnext)
                    vnext += 1
                if vnext < SC:
                    emit_vproj(vnext)
                    vnext += 1
                emit_tail(st, strips[0][0])
            while vnext < SC:
                emit_vproj(vnext)
                vnext += 1

            post_pending = None
            for qc in range(NQ):
                qTb, qTs, aoT, sums = strips[qc]
                if qc + 1 < NQ:
                    strips[qc + 1] = alloc_strip(qc + 1)
                    nxt = list(range(4 * qc + 4, 4 * qc + 8))
                else:
                    nxt = []
                npairs = 2 * qc + 2
                U = MC * npairs
                inj = {}
                for i, s_i in enumerate(nxt):
                    inj.setdefault(max(1, (U * (i + 1)) // 6), []).append(s_i)
                post_at = max(2, U // 2)
                u = 0
                for s_i in inj.pop(0, []):
                    st = emit_proj(s_i)
                    emit_tail(st, strips[qc + 1][0])
                for m in range(MC):
                    pvps = [ps_pv.tile([65, QW], F32, tag="pvp", name="pvp")
                            for _ in range(2)]
                    for t in range(0, 4 * qc + 4, 2):
                        attn_unit(qc, m, t, qTs, pvps)
                        u += 1
                        for s_i in inj.pop(u, []):
                            st = emit_proj(s_i)
                            emit_tail(st, strips[qc + 1][0])
                        if u == post_at and post_pending is not None:
                            attn_post(*post_pending)
                            post_pending = None
                    pair_end(qc, m, aoT, sums, pvps)
                for k in sorted(inj):
                    for s_i in inj[k]:
                        st = emit_proj(s_i)
                        emit_tail(st, strips[qc + 1][0])
                if post_pending is not None:
                    attn_post(*post_pending)
                post_pending = (qc, aoT, sums)
            attn_post(*post_pending)
    nc.compile()
    return nc


def _rope_tables():
    inv_freq = 1.0 / (ROPE_THETA ** (np.arange(0, D, 2, dtype=np.float32) / D))
    t = np.arange(S, dtype=np.float32)
    freqs = np.outer(t, inv_freq)                       # [S, 32]
    emb = np.concatenate([freqs, freqs], axis=-1)       # [S, 64]
    cos = np.cos(emb).astype(np.float32)
    sin = np.sin(emb).astype(np.float32)
    sin_signed = sin.copy()
    sin_signed[:, 0:32] *= -1.0                         # fold rotate_half sign
    sin_p = np.roll(sin_signed, 32, axis=1)             # pre-swap halves
    cos6 = np.tile(cos, (1, NH)).astype(np.float32)
    sin6p = np.tile(sin_p, (1, NH)).astype(np.float32)
    return np.ascontiguousarray(cos6), np.ascontiguousarray(sin6p)


_STATE = {}


def _get_program():
    if "nc" not in _STATE:
        _STATE["nc"] = build_program()
    return _STATE["nc"]


def _bf(x):
    return np.ascontiguousarray(np.asarray(x, dtype=np.float32)).astype(
        ml_dtypes.bfloat16)


def _make_in_maps(hidden_states, Wq, Wk, Wv, Wo):
    hs = np.asarray(hidden_states, dtype=np.float32)
    Wq = np.asarray(Wq, dtype=np.float32)
    Wk = np.asarray(Wk, dtype=np.float32)
    Wv = np.asarray(Wv, dtype=np.float32)
    Wo = np.asarray(Wo, dtype=np.float32)

    cos6, sin6p = _rope_tables()
    tri = np.triu(np.ones((P, P), dtype=np.float32))        # j >= i keep
    eye = np.eye(P, dtype=np.float32)
    emat = np.repeat(np.eye(NH, dtype=np.float32), D, axis=1)  # [6, 384]

    in_maps = []
    for c in range(N_CORES):
        b, g = c // 2, c % 2
        cols = slice(g * F, (g + 1) * F)
        wqk = np.concatenate([Wq[cols, :].T, Wk[cols, :].T], axis=1)
        in_maps.append({
            "xT": _bf(hs[b].T),                    # [768, S]
            "wqkT": _bf(wqk),                      # [768, 768]
            "wvT": _bf(Wv[cols, :].T),
            "woT": _bf(Wo[:, cols].T),             # [384, 768]
            "cos6": _bf(cos6),
            "sin6p": _bf(sin6p),
            "tri": _bf(tri),
            "eye": eye,
            "emat": emat.astype(np.float32),
            "ones6": _bf(np.ones((P, NH), dtype=np.float32)),
        })
    return in_maps


def run(hidden_states, Wq, Wk, Wv, Wo, trace=False, **trace_kw):
    nc = _get_program()
    in_maps = _make_in_maps(hidden_states, Wq, Wk, Wv, Wo)
    res = run_bass_kernel_spmd(nc, in_maps, core_ids=list(range(N_CORES)),
                               trace=trace, **trace_kw)
    B = 4
    out = np.empty((B, S, HID), dtype=np.float32)
    for b in range(B):
        out[b] = (res.results[2 * b]["out"].astype(np.float32)
                  + res.results[2 * b + 1]["out"].astype(np.float32))
    return out, res


def kernel(hidden_states, Wq, Wk, Wv, Wo):
    out, _ = run(hidden_states, Wq, Wk, Wv, Wo,
                 trace=bool(int(os.environ.get("KERNEL_TRACE", "0"))))
    return out

